# revision 46
# baseline (speedup 1.0000x reference)
"""Trainium2 Bass kernel for nn_AST_LSTM (GRU-based AST message passing).

Algorithm notes
---------------
The reference peels leaf edges of a random tree for 15 iterations; the
edge schedule (which edges fire when, and the compacted index remapping)
depends ONLY on E, so it is precomputed on the host. Per iteration the
device does, for each of 8 row-sharded cores:

    q = S_k @ h            (sparse mean-aggregate of gathered rows)
    G = [q | h] @ Wcat + b  with Wcat = [conv_w @ w_ih.T ; w_hh.T]
    r = sigmoid(G[:, :384]); z = sigmoid(G[:, 384:768])
    n = tanh(gi_n + b_ih_n + r * (gh_n + b_hh_n))
    h' = n + z * (h - n)

Rows are block-cyclically sharded (block=384) over 8 cores; each core
holds h transposed in SBUF as one tile [128, 10, 3, 384] (feature
partition-major) so all matmuls run directly.  Message sources are
exchanged once per iteration with an fp8 AllGather of just the needed
rows (the "halo"); iteration 0 gathers from a full local copy of V.

Precision: all nine gate output blocks (r, z, n) run in fp8e4m3
(weights pre-scaled by 16, the sigmoid/tanh un-scale via their scale
port) using DoubleRow perf mode for the first two K-blocks.  h lives
in SBUF as bf16 plus an fp8 shadow copy refreshed each iteration on
the GpSimd engine.  PSUM accumulates fp32.  r/z biases ride the
sigmoid bias port; b_hh_n (x16) is fused into the DVE (gh_n + b) * r
op; b_ih_n rides the tanh bias port (cold) or the t2 DVE op (hot), so
no bias matmuls remain.  Each window's tanh/h' tail is software-
pipelined one window behind its matmuls, cold windows run first each
iteration to cover the collective, and the AllGather fires as soon as
the last source window's h' lands.  DRAM inputs are packed partition-
major so every init DMA is 128 long descriptors.
"""
import os
import sys
import numpy as np

sys.path.insert(0, "/opt/trn_rl_repo")
import ml_dtypes

N = 30000
D = 384
NC = 8
ITERS = int(os.environ.get("KERNEL_ITERS", "15"))
B = 384              # assignment block == window width
WPC = 10             # windows per core
LROWS = B * WPC      # 3840 local rows (padded)
NBLKS = (N + B - 1) // B
OOB = 1 << 20
BF16 = ml_dtypes.bfloat16
FP8 = ml_dtypes.float8_e4m3
W8SCALE = 16.0
N8 = os.environ.get("KERNEL_N8", "0") == "1"      # fp8 n-gate
AG8 = os.environ.get("KERNEL_AG8", "1") == "1"    # fp8 collective payload
H8DMA = os.environ.get("KERNEL_H8DMA", "1") == "1"  # fp8 shadow via DMA cast

# ----------------------------------------------------------------------------
# host-side schedule
# ----------------------------------------------------------------------------

def _local_row(g):
    return ((g // B) // NC) * B + g % B


def _global_rows_of(c):
    out = np.full(LROWS, -1, dtype=np.int64)
    for w in range(WPC):
        blk = w * NC + c
        if blk >= NBLKS:
            continue
        g0 = blk * B
        n = min(B, N - g0)
        out[w * B: w * B + n] = np.arange(g0, g0 + n)
    return out


def _peel_schedule(E):
    src = np.asarray(E[0], dtype=np.int64)
    dst = np.asarray(E[1], dtype=np.int64)
    M = src.shape[0]
    active = np.ones(M, dtype=bool)
    iters = []
    for _ in range(ITERS):
        tgt_cnt = np.zeros(N, np.int64)
        np.add.at(tgt_cnt, dst, active.astype(np.int64))
        use = active & (tgt_cnt == 0)[src]
        ui = use.astype(np.int64)
        node_used = np.zeros(N, np.int64)
        np.maximum.at(node_used, src, ui)
        np.maximum.at(node_used, dst, ui)
        index_map = np.cumsum(node_used) - 1
        s_idx = index_map[src[use]]
        t_idx = index_map[dst[use]]
        cnt = np.zeros(N, np.int64)
        np.add.at(cnt, t_idx, 1)
        iters.append((s_idx, t_idx, cnt))
        active = active & ~use
    return iters


def build_schedule(E):
    """Static schedule: identical program structure for all cores, per-core
    index/matrix data (padded to union shapes)."""
    peel = _peel_schedule(E)
    its = []
    for k in range(ITERS):
        s_idx, t_idx, cnt = peel[k]
        it = {}
        # sources -> AllGather plan
        if k == 0:
            pool_pos, P, src_sched = None, 0, None
        else:
            srcs = np.unique(s_idx)
            per_core = [np.sort(srcs[(srcs // B) % NC == c]) for c in range(NC)]
            P = max(1, max(len(x) for x in per_core))
            pool_pos = {}
            swin_cb = set()
            slot_of = [dict() for _ in range(NC)]
            for c in range(NC):
                for slot, g in enumerate(per_core[c]):
                    g = int(g)
                    pool_pos[g] = c * P + slot
                    slot_of[c][g] = slot
                    lr = _local_row(g)
                    swin_cb.add((lr // B, (lr % B) // 128))
            swin_cb = sorted(swin_cb)
            src_sched = {"swin_cb": swin_cb, "slot_of": slot_of}
        it["P"] = P
        it["src"] = src_sched

        # targets -> gather blocks + aggregation matrices
        tc = (t_idx // B) % NC
        tw = (t_idx // B) // NC
        hotwins = sorted(set(tw.tolist()))
        nblk_w = {}
        for w in hotwins:
            mx = 1
            for c in range(NC):
                ne = int(((tw == w) & (tc == c)).sum())
                mx = max(mx, (ne + 127) // 128)
            nblk_w[w] = mx
        it["hotwins"] = hotwins
        it["nblk_w"] = nblk_w
        it["tw_tc"] = (tw, tc, s_idx, t_idx, cnt, pool_pos)
        its.append(it)

    # window processing order: sources for the NEXT iteration's AllGather
    # are computed as early as possible (one plain-cold window first as a
    # pipeline buffer), so the collective fires early in the iteration and
    # its transfer rides under the remaining windows' compute.
    for k, it in enumerate(its):
        if k + 1 < ITERS:
            src_wins = sorted(set(w for (w, _cb)
                                  in its[k + 1]["src"]["swin_cb"]))
        else:
            src_wins = []
        hot = set(it["hotwins"])
        src_set = set(src_wins)
        plain_cold = [w for w in range(WPC)
                      if w not in hot and w not in src_set]
        src_cold = [w for w in src_wins if w not in hot]
        src_hot = [w for w in src_wins if w in hot]
        plain_hot = [w for w in it["hotwins"] if w not in src_set]
        # iteration 0 has no incoming collective, so its source windows can
        # run immediately; later iterations keep one plain-cold window of
        # slack in front of the (gather-dependent) source windows.
        lead = plain_cold[:1] if k > 0 else []
        worder = (lead + src_cold + src_hot +
                  plain_cold[len(lead):] + plain_hot)
        it["worder"] = worder
        it["hot_order"] = [w for w in worder if w in hot]
        # scatter list in tail-consumption order (worder, then cb asc)
        if k + 1 < ITERS:
            cb_of_w = {}
            for (w, cb) in its[k + 1]["src"]["swin_cb"]:
                cb_of_w.setdefault(w, []).append(cb)
            scat = [(w, cb) for w in worder
                    for cb in sorted(cb_of_w.get(w, []))]
            slot_of = its[k + 1]["src"]["slot_of"]
            tabs = []
            for (w, cb) in scat:
                tab = np.full((NC, 128), OOB, dtype=np.int32)
                for c in range(NC):
                    blk = w * NC + c
                    if blk >= NBLKS:
                        continue
                    g0 = blk * B + cb * 128
                    for p in range(128):
                        s = slot_of[c].get(g0 + p)
                        if s is not None:
                            tab[c, p] = s
                tabs.append(tab)
            it["scat"] = scat
            it["scat_tables"] = tabs
        else:
            it["scat"] = []
            it["scat_tables"] = []

    # pack the gather/aggregation blocks in hot_order.
    for k, it in enumerate(its):
        hot_order = it["hot_order"]
        tw, tc, s_idx, t_idx, cnt, pool_pos = it.pop("tw_tc")
        nblk_w = it["nblk_w"]
        nblk_total = sum(nblk_w.values())
        gidx = np.zeros((NC, nblk_total, 128), dtype=np.int32)
        smat = np.zeros((NC, nblk_total, 128, B), dtype=np.float32)
        bpos = 0
        blocks_of_w = {}
        for w in hot_order:
            blocks_of_w[w] = (bpos, nblk_w[w])
            for c in range(NC):
                m = (tw == w) & (tc == c)
                ss, tt = s_idx[m], t_idx[m]
                order = np.argsort(tt, kind="stable")
                ss, tt = ss[order], tt[order]
                for e in range(len(ss)):
                    b = bpos + e // 128
                    p = e % 128
                    gidx[c, b, p] = ss[e] if k == 0 else pool_pos[int(ss[e])]
                    smat[c, b, p, int(tt[e]) % B] = 1.0 / cnt[tt[e]]
            bpos += nblk_w[w]
        it["blocks_of_w"] = blocks_of_w
        it["nblk_total"] = nblk_total
        it["gidx"] = gidx
        it["smat"] = smat
    return its


# ----------------------------------------------------------------------------
# bass program
# ----------------------------------------------------------------------------

def build_bass(sched):
    import concourse.bass as bass
    import concourse.bacc as bacc
    import concourse.mybir as mybir
    import concourse.tile as tile

    bf = mybir.dt.bfloat16
    f8 = mybir.dt.float8e4
    f32 = mybir.dt.float32
    i32 = mybir.dt.int32
    AF = mybir.ActivationFunctionType
    Alu = mybir.AluOpType
    DR = mybir.MatmulPerfMode.DoubleRow
    agdt = f8 if AG8 else bf

    NGB = sum(it["nblk_total"] for it in sched)
    NSB = sum(len(it["scat_tables"]) for it in sched) or 1
    NB0 = sched[0]["nblk_total"]

    nc = bacc.Bacc("TRN2", target_bir_lowering=False, debug=False,
                   enable_asserts=True, num_devices=NC)
    # partition-major packed inputs (long init DMA descriptors)
    VT0 = nc.dram_tensor("VT0", [128, WPC, 3, B], bf, kind="ExternalInput").ap()
    VT08 = nc.dram_tensor("VT08", [128, WPC, 3, B], f8,
                          kind="ExternalInput").ap()
    VF = nc.dram_tensor("VF", [N, D], bf, kind="ExternalInput").ap()
    # fp8 gate weights (x16): [128k, path(ih,hh), m(0..8 = r0-2 z0-2 n0-2
    # when N8 else 0..5), kt, 128]
    NM = 9 if N8 else 6
    # r/z weights packed kt-major as [hh kt0..2 | ih kt0..2] per output
    # block so hot windows run 3 straight DoubleRow matmuls over the
    # combined [h | q] fp8 rhs tile
    WC8 = nc.dram_tensor("WC8", [128, NM, 6, 128], f8,
                         kind="ExternalInput").ap()
    if not N8:
        WCN = nc.dram_tensor("WCN", [128, 2, 3, 3, 128], bf,
                             kind="ExternalInput").ap()
    BCOL = nc.dram_tensor("BCOL", [128, 15], f32, kind="ExternalInput").ap()
    IDN = nc.dram_tensor("IDN", [128, 128], bf, kind="ExternalInput").ap()
    GIDX = nc.dram_tensor("GIDX", [128, NGB], i32, kind="ExternalInput").ap()
    SIDX = nc.dram_tensor("SIDX", [128, NSB], i32, kind="ExternalInput").ap()
    SMAT = nc.dram_tensor("SMAT", [128, NGB, B], bf, kind="ExternalInput").ap()
    OUT = nc.dram_tensor("OUT", [WPC, 128, 3, B], bf, kind="ExternalOutput").ap()

    GBUFS = 5
    WB = 4   # work-pool rotation depth for per-window tiles

    with tile.TileContext(nc) as tc:
        with tc.tile_pool(name="const", bufs=1) as cp, \
             tc.tile_pool(name="state", bufs=1) as st, \
             tc.tile_pool(name="work", bufs=2) as wk, \
             tc.tile_pool(name="psum", bufs=2, space="PSUM") as ps, \
             tc.tile_pool(name="dram", bufs=1, space="DRAM") as dp:

            # iteration-0 window processing order -- used to order the
            # h-state init DMAs so the first windows' matmuls can start as
            # soon as possible.
            worder0 = sched[0]["worder"]

            # resident constants, in first-use order
            gidx = cp.tile([128, NGB], i32)
            nc.sync.dma_start(out=gidx[:], in_=GIDX[:])
            wc8 = cp.tile([128, NM, 6, 128], f8)
            nc.sync.dma_start(out=wc8[:], in_=WC8[:])
            # state: single-buffered transposed hidden (bf16) + fp8 shadow,
            # one tile per window so every op sees a flat contiguous AP.
            # h8 holds [h kt0..2 | q kt0..2]; the q half is (re)written by
            # each hot window's aggregation, the h half by each tail's cast
            h8 = [st.tile([128, 6, B], f8, tag=f"h8w{w}", name=f"h8w{w}")
                  for w in range(WPC)]
            hT = [st.tile([128, 3, B], bf, tag=f"hw{w}", name=f"hw{w}")
                  for w in range(WPC)]
            smat = cp.tile([128, NGB, B], bf)
            # interleave the h-shadow loads with the per-window smat slices
            # in processing order so the first window's aggregation and gate
            # matmuls unblock as early as possible
            it0blk = sched[0]["blocks_of_w"]
            for w in worder0:
                nc.sync.dma_start(out=h8[w][:, 0:3, :], in_=VT08[:, w])
                if w in it0blk:
                    bp, nb = it0blk[w]
                    nc.sync.dma_start(out=smat[:, bp:bp + nb, :],
                                      in_=SMAT[:, bp:bp + nb, :])
            bcol = cp.tile([128, 15], f32)
            nc.sync.dma_start(out=bcol[:], in_=BCOL[:])
            if not N8:
                wcn = cp.tile([128, 2, 3, 3, 128], bf)
                nc.sync.dma_start(out=wcn[:], in_=WCN[:])
            for w in worder0:
                nc.sync.dma_start(out=hT[w][:], in_=VT0[:, w])
            idn = cp.tile([128, 128], bf)
            nc.sync.dma_start(out=idn[:], in_=IDN[:])
            sidx = cp.tile([128, NSB], i32)
            nc.sync.dma_start(out=sidx[:], in_=SIDX[:])
            if NB0 < NGB:
                nc.sync.dma_start(out=smat[:, NB0:NGB, :],
                                  in_=SMAT[:, NB0:NGB, :])

            gpos = 0
            spos = 0
            deferred = [None]
            pend = {}
            qtiles = {}
            # gathers for iteration k+1 are emitted after iteration k's
            # last aggregation, so one iteration's worth of buffers is
            # enough
            GPF = max(it["nblk_total"] for it in sched)

            def emit_gathers(k, src_ap_k):
                nonlocal gpos
                tiles = []
                nb_tot = sched[k]["nblk_total"]
                for _ in range(nb_tot):
                    xg = wk.tile([128, D], bf, tag="xg", bufs=GPF, name="xg")
                    nc.gpsimd.indirect_dma_start(
                        out=xg[:], out_offset=None, in_=src_ap_k[:],
                        in_offset=bass.IndirectOffsetOnAxis(
                            ap=gidx[:, gpos:gpos + 1], axis=0))
                    tiles.append((xg, gpos))
                    gpos += 1
                return tiles

            # prefetch iteration-0 gathers so phase A starts during init DMAs
            pend[0] = emit_gathers(0, VF)

            for k in range(ITERS):
                it = sched[k]
                src_cbs = {}
                for (w, cb) in it["scat"]:
                    src_cbs.setdefault(w, []).append(cb)
                if k + 1 < ITERS:
                    P1 = sched[k + 1]["P"]
                    agin = dp.tile([P1, D], agdt, tag=f"agin{k+1}",
                                   name=f"agin{k+1}")
                    agout = dp.tile([NC * P1, D], agdt, tag=f"agout{k+1}",
                                    name=f"agout{k+1}", addr_space="Shared")

                # phase A helper: aggregate one hot window's gathered blocks
                # (emitted just before that window's stage1 so the PE queue
                # never head-of-line-blocks on not-yet-arrived gathers)
                def window_agg(w, it=it, k=k):
                    pending = pend[k]
                    bpos, nb = it["blocks_of_w"][w]
                    qp = ps.tile([128, 3, 512], f32, tag="qp",
                                 space="PSUM", name="qp", bufs=1)
                    for bi in range(nb):
                        xg, gp_i = pending[bpos + bi]
                        for kt in range(3):
                            nc.tensor.matmul(
                                qp[:, kt, :B],
                                lhsT=xg[:, kt * 128:(kt + 1) * 128],
                                rhs=smat[:, gp_i, :],
                                start=(bi == 0), stop=(bi == nb - 1))
                    nc.scalar.activation(h8[w][:, 3:6, :], qp[:, 0:3, :B],
                                         AF.Identity)
                    qt = None
                    if not N8:
                        qt = wk.tile([128, 3, B], bf, tag="q", bufs=4,
                                     name="qt")
                        nc.vector.tensor_copy(qt[:], qp[:, 0:3, :B])
                    qtiles[w] = qt

                # phase B, one window at a time; each window's tanh/h' tail
                # is deferred until after the next window's matmul stage so
                # the ACT/DVE FIFOs never head-of-line-block the PE.  Cold
                # windows run first (covering the previous AllGather), hot
                # windows follow in hot_order (next-iteration-cold first so
                # their tails land early for the next boundary's cover).
                hot = set(it["hotwins"])
                worder = it["worder"]
                src_wins = [w for w in worder if w in src_cbs]
                last_src_w = src_wins[-1] if src_wins else None
                agg_done = set()
                for w in worder:
                    # if the deferred tail writes this window's h, it must
                    # land before this window's stage1 reads it
                    if deferred[0] is not None and deferred[0][1] == w:
                        deferred[0][0]()
                        deferred[0] = None
                    if w in hot and w not in agg_done:
                        window_agg(w)
                        agg_done.add(w)
                    whot = w in hot
                    hq = hT[w][:]
                    h8q = h8[w][:]

                    def gate_group(m, with_q):
                        # fp8 gate block m.  cold: DoubleRow(h0,h1) + plain
                        # h2; hot: 3 DoubleRow matmuls straight through the
                        # combined [h0..2 | q0..2] rhs
                        gp = ps.tile([128, 512], f32, tag="gg",
                                     space="PSUM", name="gp", bufs=GBUFS)
                        nc.tensor.matmul(
                            gp[:, :B],
                            lhsT=wc8[:, m, 0:2, :],
                            rhs=h8q[:, 0:2, :],
                            start=True, stop=False, perf_mode=DR)
                        if with_q:
                            nc.tensor.matmul(
                                gp[:, :B],
                                lhsT=wc8[:, m, 2:4, :],
                                rhs=h8q[:, 2:4, :],
                                start=False, stop=False, perf_mode=DR)
                            nc.tensor.matmul(
                                gp[:, :B],
                                lhsT=wc8[:, m, 4:6, :],
                                rhs=h8q[:, 4:6, :],
                                start=False, stop=True, perf_mode=DR)
                        else:
                            nc.tensor.matmul(
                                gp[:, :B],
                                lhsT=wc8[:, m, 2, :],
                                rhs=h8q[:, 2, :],
                                start=False, stop=True)
                        return gp

                    def n_group_bf(j, path, rhs):
                        # bf16 n-gate block j for one path (0=ih/q, 1=hh/h)
                        gp = ps.tile([128, 512], f32, tag="gg",
                                     space="PSUM", name="gp", bufs=GBUFS)
                        for kt in range(3):
                            nc.tensor.matmul(
                                gp[:, :B],
                                lhsT=wcn[:, path, j, kt, :],
                                rhs=rhs[:, kt, :],
                                start=(kt == 0), stop=(kt == 2))
                        return gp

                    def n_group_f8(j, path, rhs):
                        # path 0 = ih (kt 3..5), 1 = hh (kt 0..2)
                        k0 = 0 if path == 1 else 3
                        gp = ps.tile([128, 512], f32, tag="gg",
                                     space="PSUM", name="gp", bufs=GBUFS)
                        nc.tensor.matmul(
                            gp[:, :B],
                            lhsT=wc8[:, 6 + j, k0:k0 + 2, :],
                            rhs=rhs[:, k0:k0 + 2, :],
                            start=True, stop=False, perf_mode=DR)
                        nc.tensor.matmul(
                            gp[:, :B],
                            lhsT=wc8[:, 6 + j, k0 + 2, :],
                            rhs=rhs[:, k0 + 2, :],
                            start=False, stop=True)
                        return gp

                    r_sb = wk.tile([128, 3, B], bf, tag="r", bufs=WB,
                                   name="r_sb")
                    z_sb = wk.tile([128, 3, B], bf, tag="z", bufs=WB,
                                   name="z_sb")
                    for j in range(3):
                        rp = gate_group(j, whot)
                        nc.scalar.activation(r_sb[:, j, :], rp[:, :B],
                                             AF.Sigmoid, bias=bcol[:, j:j + 1],
                                             scale=1.0 / W8SCALE)
                    for j in range(3):
                        zp = gate_group(3 + j, whot)
                        nc.scalar.activation(z_sb[:, j, :], zp[:, :B],
                                             AF.Sigmoid,
                                             bias=bcol[:, 3 + j:4 + j],
                                             scale=1.0 / W8SCALE)
                    t2 = wk.tile([128, 3, B], bf, tag="t2", bufs=WB,
                                 name="t2")
                    for j in range(3):
                        if N8:
                            hp = n_group_f8(j, 1, h8q)
                        else:
                            hp = n_group_bf(j, 1, hq)
                        if whot:
                            # t1 = (gh_n*16 + 16*b_hh_n) * r, then
                            # t2 = (gi_n*16 + 16*b_ih_n) + t1 (all x16; the
                            # tanh un-scales); cold: t2 = t1, bias rides tanh
                            t1 = wk.tile([128, B], bf, tag="t1", bufs=WB,
                                         name="t1")
                            nc.vector.scalar_tensor_tensor(
                                out=t1[:], in0=hp[:, :B],
                                scalar=bcol[:, 6 + j:7 + j],
                                in1=r_sb[:, j, :],
                                op0=Alu.add, op1=Alu.mult)
                            if N8:
                                ip = n_group_f8(j, 0, h8q)
                            else:
                                ip = n_group_bf(j, 0, qtiles[w])
                            nc.vector.scalar_tensor_tensor(
                                out=t2[:, j, :], in0=ip[:, :B],
                                scalar=bcol[:, 9 + j:10 + j],
                                in1=t1[:],
                                op0=Alu.add, op1=Alu.add)
                        else:
                            nc.vector.scalar_tensor_tensor(
                                out=t2[:, j, :], in0=hp[:, :B],
                                scalar=bcol[:, 6 + j:7 + j],
                                in1=r_sb[:, j, :],
                                op0=Alu.add, op1=Alu.mult)

                    def tail_math(w=w, hq=hq, h8q=h8q, t2=t2, z_sb=z_sb,
                                  k=k, whot=whot):
                        n_sb = wk.tile([128, 3, B], bf, tag="n", bufs=WB,
                                       name="n_sb")
                        d_sb = wk.tile([128, 3, B], bf, tag="d", bufs=WB,
                                       name="d_sb")
                        e_sb = wk.tile([128, 3, B], bf, tag="e", bufs=WB,
                                       name="e_sb")
                        tsc = 1.0 / W8SCALE if N8 else 1.0
                        if whot:
                            # bias already folded into t2 by the stt
                            nc.scalar.activation(n_sb[:], t2[:], AF.Tanh,
                                                 scale=tsc)
                        else:
                            for j in range(3):
                                nc.scalar.activation(
                                    n_sb[:, j, :], t2[:, j, :],
                                    AF.Tanh, bias=bcol[:, 12 + j:13 + j],
                                    scale=tsc)
                        nc.vector.tensor_sub(out=d_sb[:], in0=hq, in1=n_sb[:])
                        nc.vector.tensor_mul(out=e_sb[:], in0=z_sb[:],
                                             in1=d_sb[:])
                        nc.vector.tensor_add(out=hq, in0=n_sb[:],
                                             in1=e_sb[:])
                        if k < ITERS - 1:
                            if H8DMA:
                                # fp8 shadow refresh as a casting SBUF->SBUF
                                # DMA (on the DMA engines, not an ALU)
                                nc.gpsimd.dma_start(out=h8q[:, 0:3, :],
                                                    in_=hq)
                            else:
                                nc.vector.tensor_copy(h8q[:, 0:3, :], hq)
                        else:
                            nc.sync.dma_start(out=OUT[w], in_=hq)

                    def tail_export(w=w, hq=hq, src_cbs=src_cbs,
                                    fire=(w == last_src_w),
                                    agin=agin if k + 1 < ITERS else None,
                                    agout=agout if k + 1 < ITERS else None,
                                    P1=P1 if k + 1 < ITERS else None):
                        nonlocal spos
                        for cb in src_cbs.get(w, []):
                            tp = ps.tile([128, B], bf, tag="gg",
                                         space="PSUM", name="tp",
                                         bufs=GBUFS)
                            for kt in range(3):
                                nc.tensor.transpose(
                                    tp[:, kt * 128:(kt + 1) * 128],
                                    hq[:, kt, cb * 128:(cb + 1) * 128],
                                    idn[:])
                            rm = wk.tile([128, D], agdt, tag="rm", bufs=8)
                            nc.vector.tensor_copy(rm[:], tp[:])
                            nc.gpsimd.indirect_dma_start(
                                out=agin[:],
                                out_offset=bass.IndirectOffsetOnAxis(
                                    ap=sidx[:, spos:spos + 1], axis=0),
                                in_=rm[:], in_offset=None,
                                bounds_check=P1 - 1, oob_is_err=False)
                            spos += 1
                        if fire:
                            nc.gpsimd.collective_compute(
                                "AllGather", Alu.bypass,
                                replica_groups=[list(range(NC))],
                                ins=[agin[:].opt()],
                                outs=[agout[:].opt()])

                    if w in src_cbs:
                        # source windows: h' math is emitted eagerly so the
                        # tanh/DVE chain overlaps this window's own matmul
                        # drain; only the transpose/scatter half is deferred
                        # behind the next window's matmuls.
                        tail_math()
                        this_tail = tail_export
                    else:
                        def this_tail(tm=tail_math, te=tail_export):
                            tm()
                            te()
                    if deferred[0] is not None:
                        deferred[0][0]()
                    deferred[0] = (this_tail, w)

                # next iteration's gathers go on the gpsimd queue only after
                # every scatter/cast of this iteration, so their wait on the
                # (in-flight) AllGather never blocks this iteration's tail
                # work; the collective is long done when these execute.
                if k + 1 < ITERS and last_src_w is not None:
                    pend[k + 1] = emit_gathers(k + 1, agout)

            if deferred[0] is not None:
                deferred[0][0]()
                deferred[0] = None
    nc.compile()
    return nc


# ----------------------------------------------------------------------------
# host packing + entry point
# ----------------------------------------------------------------------------

def pack_inputs(sched, c, V, conv_weight, w_ih, w_hh, b_ih, b_hh):
    V = np.asarray(V, dtype=np.float32)
    Wcat = np.concatenate([np.asarray(conv_weight) @ np.asarray(w_ih).T,
                           np.asarray(w_hh).T], axis=0).astype(np.float32)
    b_ih = np.asarray(b_ih, dtype=np.float32)
    b_hh = np.asarray(b_hh, dtype=np.float32)

    grows = _global_rows_of(c)
    hl = np.zeros((LROWS, D), dtype=np.float32)
    valid = grows >= 0
    hl[valid] = V[grows[valid]]
    # VT0[p, w, kt, j] = h[w*B + j, kt*128 + p]  (partition-major)
    vt0f = np.ascontiguousarray(
        hl.reshape(WPC, B, 3, 128).transpose(3, 0, 2, 1))
    vt0 = vt0f.astype(BF16)
    vt08 = vt0f.astype(BF16).astype(FP8)
    # WC8[p, m, kt6, :]: kt6 0..2 -> hh weights (Wcat rows 384+), kt6 3..5
    # -> ih weights (Wcat rows 0..383); m = output block 0..8, x16 scale
    NM = 9 if N8 else 6
    wc8 = np.zeros((128, NM, 6, 128), dtype=np.float32)
    wcn = np.zeros((128, 2, 3, 3, 128), dtype=np.float32)
    for path in range(2):
        for kt in range(3):
            krow = (path * 3 + kt) * 128
            kt6 = kt if path == 1 else 3 + kt
            for m in range(NM):
                wc8[:, m, kt6, :] = (
                    Wcat[krow:krow + 128, m * 128:(m + 1) * 128] * W8SCALE)
            for j in range(3):
                mm = 6 + j
                wcn[:, path, j, kt, :] = \
                    Wcat[krow:krow + 128, mm * 128:(mm + 1) * 128]
    bsum = b_ih + b_hh
    bl = np.zeros((15, 128), dtype=np.float32)
    for m in range(6):
        bl[m] = bsum[m * 128:(m + 1) * 128]
    for j in range(3):
        bhn = b_hh[768 + j * 128: 768 + (j + 1) * 128]
        bin_ = b_ih[768 + j * 128: 768 + (j + 1) * 128]
        if N8:
            bl[6 + j] = bhn * W8SCALE
            bl[9 + j] = bin_ * W8SCALE
        else:
            bl[6 + j] = bhn
            bl[9 + j] = bin_
        bl[12 + j] = bin_
    bc = np.ascontiguousarray(bl.T)  # [128, 15]

    gidx = np.concatenate([it["gidx"][c] for it in sched], axis=0)  # [NGB,128]
    smat = np.concatenate([it["smat"][c] for it in sched], axis=0)  # [NGB,128,B]
    sc = [tab[c] for it in sched for tab in it["scat_tables"]]
    sidx = (np.stack(sc, axis=0) if sc else np.zeros((1, 128), np.int32))

    out = {
        "VT0": vt0,
        "VT08": vt08,
        "VF": V.astype(BF16),
        "WC8": wc8.astype(FP8),
        "BCOL": bc.astype(np.float32),
        "IDN": np.eye(128, dtype=np.float32).astype(BF16),
        "GIDX": np.ascontiguousarray(gidx.T).astype(np.int32),
        "SIDX": np.ascontiguousarray(sidx.T).astype(np.int32),
        # SMAT[p, blk, j] = smat[blk, p, j]  (partition-major)
        "SMAT": np.ascontiguousarray(smat.transpose(1, 0, 2)).astype(BF16),
    }
    if not N8:
        out["WCN"] = wcn.astype(BF16)
    return out


def unpack_output(results):
    out = np.zeros((N, D), dtype=np.float32)
    for c in range(NC):
        o = np.asarray(results[c]["OUT"], dtype=np.float32)  # [WPC,128,3,B]
        hl = o.transpose(0, 3, 2, 1).reshape(LROWS, D)
        grows = _global_rows_of(c)
        valid = grows >= 0
        out[grows[valid]] = hl[valid]
    return out


_CACHE = {}


def _install_profile_hook():
    """The agent image lacks ``antenv.axon_hooks``; shim it so
    run_bass_kernel_spmd(trace=True) can capture NTFF profiles."""
    import types
    try:
        from antenv.axon_hooks import get_axon_ntff_profile_hook  # noqa: F401
        return True
    except ImportError:
        pass
    try:
        import antenv
        from trn_agent_boot.trn_boot import _ntff_profile_via_ctypes
        hook = _ntff_profile_via_ctypes("/opt/axon/libaxon_pjrt.so")
        mod = types.ModuleType("antenv.axon_hooks")
        mod._hook = hook
        mod.set_axon_ntff_profile_hook = lambda h: setattr(mod, "_hook", h)
        mod.get_axon_ntff_profile_hook = lambda: mod._hook
        sys.modules["antenv.axon_hooks"] = mod
        antenv.axon_hooks = mod
        return hook is not None
    except Exception:
        return False


def kernel(V, E, conv_weight, w_ih, w_hh, b_ih, b_hh, _want_results=False):
    from concourse import bass_utils
    E_np = np.asarray(E)
    sched = build_schedule(E_np)
    key = tuple((it["nblk_total"], it["P"], tuple(it["hotwins"]),
                 tuple(it["src"]["swin_cb"]) if it["src"] else ())
                for it in sched)
    if key not in _CACHE:
        _CACHE[key] = build_bass(sched)
    nc = _CACHE[key]
    in_maps = [pack_inputs(sched, c, V, conv_weight, w_ih, w_hh, b_ih, b_hh)
               for c in range(NC)]
    trace = os.environ.get("KERNEL_TRACE", "0") == "1"
    if trace:
        trace = _install_profile_hook()
        # artifact upload to the fish bucket is unavailable here; stub it
        bass_utils.upload_artifacts = lambda tmpdir: "local://" + str(tmpdir)
    res = bass_utils.run_bass_kernel_spmd(
        nc, in_maps, core_ids=list(range(NC)), trace=trace,
        tmpdir=os.environ.get("KERNEL_TMPDIR"))
    out = unpack_output(res.results).astype(np.float32)
    if _want_results:
        return out, res
    return out


# revision 47
# speedup vs baseline: 1.1030x; 1.1030x over previous
"""Trainium2 Bass kernel for nn_AST_LSTM (GRU-based AST message passing).

Algorithm notes
---------------
The reference peels leaf edges of a random tree for 15 iterations; the
edge schedule (which edges fire when, and the compacted index remapping)
depends ONLY on E, so it is precomputed on the host. Per iteration the
device does, for each of 8 row-sharded cores:

    q = S_k @ h            (sparse mean-aggregate of gathered rows)
    G = [q | h] @ Wcat + b  with Wcat = [conv_w @ w_ih.T ; w_hh.T]
    r = sigmoid(G[:, :384]); z = sigmoid(G[:, 384:768])
    n = tanh(gi_n + b_ih_n + r * (gh_n + b_hh_n))
    h' = n + z * (h - n)

Rows are block-cyclically sharded (block=384) over 8 cores; each core
holds h transposed in SBUF as one tile [128, 10, 3, 384] (feature
partition-major) so all matmuls run directly.  Message sources are
exchanged once per iteration with an fp8 AllGather of just the needed
rows (the "halo"); iteration 0 gathers from a full local copy of V.

Precision: all nine gate output blocks (r, z, n) run in fp8e4m3
(weights pre-scaled by 16, the sigmoid/tanh un-scale via their scale
port) using DoubleRow perf mode for the first two K-blocks.  h lives
in SBUF as bf16 plus an fp8 shadow copy refreshed each iteration on
the GpSimd engine.  PSUM accumulates fp32.  r/z biases ride the
sigmoid bias port; b_hh_n (x16) is fused into the DVE (gh_n + b) * r
op; b_ih_n rides the tanh bias port (cold) or the t2 DVE op (hot), so
no bias matmuls remain.  Each window's tanh/h' tail is software-
pipelined one window behind its matmuls, cold windows run first each
iteration to cover the collective, and the AllGather fires as soon as
the last source window's h' lands.  DRAM inputs are packed partition-
major so every init DMA is 128 long descriptors.
"""
import os
import sys
import numpy as np

sys.path.insert(0, "/opt/trn_rl_repo")
import ml_dtypes

N = 30000
D = 384
NC = 8
ITERS = int(os.environ.get("KERNEL_ITERS", "15"))
B = 384              # assignment block == window width
WPC = 10             # windows per core
LROWS = B * WPC      # 3840 local rows (padded)
NBLKS = (N + B - 1) // B
OOB = 1 << 20
BF16 = ml_dtypes.bfloat16
FP8 = ml_dtypes.float8_e4m3
W8SCALE = 16.0
N8 = os.environ.get("KERNEL_N8", "0") == "1"      # fp8 n-gate
AG8 = os.environ.get("KERNEL_AG8", "1") == "1"    # fp8 collective payload
H8DMA = os.environ.get("KERNEL_H8DMA", "1") == "1"  # fp8 shadow via DMA cast

# ----------------------------------------------------------------------------
# host-side schedule
# ----------------------------------------------------------------------------

def _local_row(g):
    return ((g // B) // NC) * B + g % B


def _global_rows_of(c):
    out = np.full(LROWS, -1, dtype=np.int64)
    for w in range(WPC):
        blk = w * NC + c
        if blk >= NBLKS:
            continue
        g0 = blk * B
        n = min(B, N - g0)
        out[w * B: w * B + n] = np.arange(g0, g0 + n)
    return out


def _peel_schedule(E):
    src = np.asarray(E[0], dtype=np.int64)
    dst = np.asarray(E[1], dtype=np.int64)
    M = src.shape[0]
    active = np.ones(M, dtype=bool)
    iters = []
    for _ in range(ITERS):
        tgt_cnt = np.zeros(N, np.int64)
        np.add.at(tgt_cnt, dst, active.astype(np.int64))
        use = active & (tgt_cnt == 0)[src]
        ui = use.astype(np.int64)
        node_used = np.zeros(N, np.int64)
        np.maximum.at(node_used, src, ui)
        np.maximum.at(node_used, dst, ui)
        index_map = np.cumsum(node_used) - 1
        s_idx = index_map[src[use]]
        t_idx = index_map[dst[use]]
        cnt = np.zeros(N, np.int64)
        np.add.at(cnt, t_idx, 1)
        iters.append((s_idx, t_idx, cnt))
        active = active & ~use
    return iters


def build_schedule(E):
    """Static schedule: identical program structure for all cores, per-core
    index/matrix data (padded to union shapes)."""
    peel = _peel_schedule(E)
    its = []
    for k in range(ITERS):
        s_idx, t_idx, cnt = peel[k]
        it = {}
        # sources -> AllGather plan
        if k == 0:
            pool_pos, P, src_sched = None, 0, None
        else:
            srcs = np.unique(s_idx)
            per_core = [np.sort(srcs[(srcs // B) % NC == c]) for c in range(NC)]
            P = max(1, max(len(x) for x in per_core))
            pool_pos = {}
            swin_cb = set()
            slot_of = [dict() for _ in range(NC)]
            for c in range(NC):
                for slot, g in enumerate(per_core[c]):
                    g = int(g)
                    pool_pos[g] = c * P + slot
                    slot_of[c][g] = slot
                    lr = _local_row(g)
                    swin_cb.add((lr // B, (lr % B) // 128))
            swin_cb = sorted(swin_cb)
            src_sched = {"swin_cb": swin_cb, "slot_of": slot_of}
        it["P"] = P
        it["src"] = src_sched

        # targets -> gather blocks + aggregation matrices
        tc = (t_idx // B) % NC
        tw = (t_idx // B) // NC
        hotwins = sorted(set(tw.tolist()))
        nblk_w = {}
        for w in hotwins:
            mx = 1
            for c in range(NC):
                ne = int(((tw == w) & (tc == c)).sum())
                mx = max(mx, (ne + 127) // 128)
            nblk_w[w] = mx
        it["hotwins"] = hotwins
        it["nblk_w"] = nblk_w
        it["tw_tc"] = (tw, tc, s_idx, t_idx, cnt, pool_pos)
        its.append(it)

    # window processing order: sources for the NEXT iteration's AllGather
    # are computed as early as possible (one plain-cold window first as a
    # pipeline buffer), so the collective fires early in the iteration and
    # its transfer rides under the remaining windows' compute.
    for k, it in enumerate(its):
        if k + 1 < ITERS:
            src_wins = sorted(set(w for (w, _cb)
                                  in its[k + 1]["src"]["swin_cb"]))
        else:
            src_wins = []
        hot = set(it["hotwins"])
        src_set = set(src_wins)
        plain_cold = [w for w in range(WPC)
                      if w not in hot and w not in src_set]
        src_cold = [w for w in src_wins if w not in hot]
        src_hot = [w for w in src_wins if w in hot]
        plain_hot = [w for w in it["hotwins"] if w not in src_set]
        # iteration 0 has no incoming collective, so its source windows can
        # run immediately; later iterations keep one plain-cold window of
        # slack in front of the (gather-dependent) source windows.
        lead = plain_cold[:1] if k > 0 else []
        worder = (lead + src_cold + src_hot +
                  plain_cold[len(lead):] + plain_hot)
        it["worder"] = worder
        it["hot_order"] = [w for w in worder if w in hot]
        # scatter list in tail-consumption order (worder, then cb asc)
        if k + 1 < ITERS:
            cb_of_w = {}
            for (w, cb) in its[k + 1]["src"]["swin_cb"]:
                cb_of_w.setdefault(w, []).append(cb)
            scat = [(w, cb) for w in worder
                    for cb in sorted(cb_of_w.get(w, []))]
            slot_of = its[k + 1]["src"]["slot_of"]
            tabs = []
            for (w, cb) in scat:
                tab = np.full((NC, 128), OOB, dtype=np.int32)
                for c in range(NC):
                    blk = w * NC + c
                    if blk >= NBLKS:
                        continue
                    g0 = blk * B + cb * 128
                    for p in range(128):
                        s = slot_of[c].get(g0 + p)
                        if s is not None:
                            tab[c, p] = s
                tabs.append(tab)
            it["scat"] = scat
            it["scat_tables"] = tabs
        else:
            it["scat"] = []
            it["scat_tables"] = []

    # pack the gather/aggregation blocks in hot_order.
    for k, it in enumerate(its):
        hot_order = it["hot_order"]
        tw, tc, s_idx, t_idx, cnt, pool_pos = it.pop("tw_tc")
        nblk_w = it["nblk_w"]
        nblk_total = sum(nblk_w.values())
        gidx = np.zeros((NC, nblk_total, 128), dtype=np.int32)
        smat = np.zeros((NC, nblk_total, 128, B), dtype=np.float32)
        bpos = 0
        blocks_of_w = {}
        for w in hot_order:
            blocks_of_w[w] = (bpos, nblk_w[w])
            for c in range(NC):
                m = (tw == w) & (tc == c)
                ss, tt = s_idx[m], t_idx[m]
                order = np.argsort(tt, kind="stable")
                ss, tt = ss[order], tt[order]
                for e in range(len(ss)):
                    b = bpos + e // 128
                    p = e % 128
                    gidx[c, b, p] = ss[e] if k == 0 else pool_pos[int(ss[e])]
                    smat[c, b, p, int(tt[e]) % B] = 1.0 / cnt[tt[e]]
            bpos += nblk_w[w]
        it["blocks_of_w"] = blocks_of_w
        it["nblk_total"] = nblk_total
        it["gidx"] = gidx
        it["smat"] = smat
    return its


# ----------------------------------------------------------------------------
# bass program
# ----------------------------------------------------------------------------

def build_bass(sched):
    import concourse.bass as bass
    import concourse.bacc as bacc
    import concourse.mybir as mybir
    import concourse.tile as tile

    bf = mybir.dt.bfloat16
    f8 = mybir.dt.float8e4
    f32 = mybir.dt.float32
    i32 = mybir.dt.int32
    AF = mybir.ActivationFunctionType
    Alu = mybir.AluOpType
    DR = mybir.MatmulPerfMode.DoubleRow
    agdt = f8 if AG8 else bf

    NGB = sum(it["nblk_total"] for it in sched)
    NSB = sum(len(it["scat_tables"]) for it in sched) or 1
    NB0 = sched[0]["nblk_total"]

    nc = bacc.Bacc("TRN2", target_bir_lowering=False, debug=False,
                   enable_asserts=True, num_devices=NC)
    # partition-major packed inputs (long init DMA descriptors)
    VT0 = nc.dram_tensor("VT0", [128, WPC, 3, B], bf, kind="ExternalInput").ap()
    VT08 = nc.dram_tensor("VT08", [128, WPC, 3, B], f8,
                          kind="ExternalInput").ap()
    VF = nc.dram_tensor("VF", [N, D], bf, kind="ExternalInput").ap()
    # fp8 gate weights (x16): [128k, path(ih,hh), m(0..8 = r0-2 z0-2 n0-2
    # when N8 else 0..5), kt, 128]
    NM = 9 if N8 else 6
    # r/z weights packed kt-major as [hh kt0..2 | ih kt0..2] per output
    # block so hot windows run 3 straight DoubleRow matmuls over the
    # combined [h | q] fp8 rhs tile
    WC8 = nc.dram_tensor("WC8", [128, NM, 6, 128], f8,
                         kind="ExternalInput").ap()
    if not N8:
        WCN = nc.dram_tensor("WCN", [128, 2, 3, 3, 128], bf,
                             kind="ExternalInput").ap()
    BCOL = nc.dram_tensor("BCOL", [128, 15], f32, kind="ExternalInput").ap()
    IDN = nc.dram_tensor("IDN", [128, 128], bf, kind="ExternalInput").ap()
    GIDX = nc.dram_tensor("GIDX", [128, NGB], i32, kind="ExternalInput").ap()
    SIDX = nc.dram_tensor("SIDX", [128, NSB], i32, kind="ExternalInput").ap()
    SMAT = nc.dram_tensor("SMAT", [128, NGB, B], bf, kind="ExternalInput").ap()
    OUT = nc.dram_tensor("OUT", [WPC, 128, 3, B], bf, kind="ExternalOutput").ap()

    GBUFS = 5
    WB = 4   # work-pool rotation depth for per-window tiles

    with tile.TileContext(nc) as tc:
        with tc.tile_pool(name="const", bufs=1) as cp, \
             tc.tile_pool(name="state", bufs=1) as st, \
             tc.tile_pool(name="work", bufs=2) as wk, \
             tc.tile_pool(name="psum", bufs=2, space="PSUM") as ps, \
             tc.tile_pool(name="dram", bufs=1, space="DRAM") as dp:

            # iteration-0 window processing order -- used to order the
            # h-state init DMAs so the first windows' matmuls can start as
            # soon as possible.
            worder0 = sched[0]["worder"]

            # resident constants, in first-use order
            gidx = cp.tile([128, NGB], i32)
            nc.sync.dma_start(out=gidx[:], in_=GIDX[:])
            wc8 = cp.tile([128, NM, 6, 128], f8)
            nc.sync.dma_start(out=wc8[:], in_=WC8[:])
            # state: single-buffered transposed hidden (bf16) + fp8 shadow,
            # one tile per window so every op sees a flat contiguous AP.
            # h8 holds [h kt0..2 | q kt0..2]; the q half is (re)written by
            # each hot window's aggregation, the h half by each tail's cast
            h8 = [st.tile([128, 6, B], f8, tag=f"h8w{w}", name=f"h8w{w}")
                  for w in range(WPC)]
            hT = [st.tile([128, 3, B], bf, tag=f"hw{w}", name=f"hw{w}")
                  for w in range(WPC)]
            smat = cp.tile([128, NGB, B], bf)
            # interleave the h-shadow loads with the per-window smat slices
            # in processing order so the first window's aggregation and gate
            # matmuls unblock as early as possible
            it0blk = sched[0]["blocks_of_w"]
            for w in worder0:
                nc.sync.dma_start(out=h8[w][:, 0:3, :], in_=VT08[:, w])
                if w in it0blk:
                    bp, nb = it0blk[w]
                    nc.sync.dma_start(out=smat[:, bp:bp + nb, :],
                                      in_=SMAT[:, bp:bp + nb, :])
            bcol = cp.tile([128, 15], f32)
            nc.sync.dma_start(out=bcol[:], in_=BCOL[:])
            if not N8:
                wcn = cp.tile([128, 2, 3, 3, 128], bf)
                nc.sync.dma_start(out=wcn[:], in_=WCN[:])
            for w in worder0:
                nc.sync.dma_start(out=hT[w][:], in_=VT0[:, w])
            idn = cp.tile([128, 128], bf)
            nc.sync.dma_start(out=idn[:], in_=IDN[:])
            sidx = cp.tile([128, NSB], i32)
            nc.sync.dma_start(out=sidx[:], in_=SIDX[:])
            if NB0 < NGB:
                nc.sync.dma_start(out=smat[:, NB0:NGB, :],
                                  in_=SMAT[:, NB0:NGB, :])

            gpos = 0
            spos = 0
            deferred = [None]
            pend = {}
            qtiles = {}
            # gathers for iteration k+1 are emitted after iteration k's
            # last aggregation, so one iteration's worth of buffers is
            # enough
            GPF = max(it["nblk_total"] for it in sched)

            def emit_gathers(k, src_ap_k):
                nonlocal gpos
                tiles = []
                nb_tot = sched[k]["nblk_total"]
                for _ in range(nb_tot):
                    xg = wk.tile([128, D], bf, tag="xg", bufs=GPF, name="xg")
                    nc.gpsimd.indirect_dma_start(
                        out=xg[:], out_offset=None, in_=src_ap_k[:],
                        in_offset=bass.IndirectOffsetOnAxis(
                            ap=gidx[:, gpos:gpos + 1], axis=0))
                    tiles.append((xg, gpos))
                    gpos += 1
                return tiles

            # prefetch iteration-0 gathers so phase A starts during init DMAs
            pend[0] = emit_gathers(0, VF)

            for k in range(ITERS):
                it = sched[k]
                src_cbs = {}
                for (w, cb) in it["scat"]:
                    src_cbs.setdefault(w, []).append(cb)
                if k + 1 < ITERS:
                    P1 = sched[k + 1]["P"]
                    agin = dp.tile([P1, D], agdt, tag=f"agin{k+1}",
                                   name=f"agin{k+1}")
                    agout = dp.tile([NC * P1, D], agdt, tag=f"agout{k+1}",
                                    name=f"agout{k+1}", addr_space="Shared")

                # phase A helper: aggregate one hot window's gathered blocks
                # (emitted just before that window's stage1 so the PE queue
                # never head-of-line-blocks on not-yet-arrived gathers)
                def window_agg(w, it=it, k=k):
                    pending = pend[k]
                    bpos, nb = it["blocks_of_w"][w]
                    qp = ps.tile([128, 3, 512], f32, tag="qp",
                                 space="PSUM", name="qp", bufs=1)
                    for bi in range(nb):
                        xg, gp_i = pending[bpos + bi]
                        for kt in range(3):
                            nc.tensor.matmul(
                                qp[:, kt, :B],
                                lhsT=xg[:, kt * 128:(kt + 1) * 128],
                                rhs=smat[:, gp_i, :],
                                start=(bi == 0), stop=(bi == nb - 1))
                    nc.scalar.activation(h8[w][:, 3:6, :], qp[:, 0:3, :B],
                                         AF.Identity)
                    qt = None
                    if not N8:
                        qt = wk.tile([128, 3, B], bf, tag="q", bufs=4,
                                     name="qt")
                        nc.vector.tensor_copy(qt[:], qp[:, 0:3, :B])
                    qtiles[w] = qt

                # phase B, one window at a time; each window's tanh/h' tail
                # is deferred until after the next window's matmul stage so
                # the ACT/DVE FIFOs never head-of-line-block the PE.  Cold
                # windows run first (covering the previous AllGather), hot
                # windows follow in hot_order (next-iteration-cold first so
                # their tails land early for the next boundary's cover).
                hot = set(it["hotwins"])
                worder = it["worder"]
                src_wins = [w for w in worder if w in src_cbs]
                last_src_w = src_wins[-1] if src_wins else None
                agg_done = set()
                for w in worder:
                    # if the deferred tail writes this window's h, it must
                    # land before this window's stage1 reads it
                    if deferred[0] is not None and deferred[0][1] == w:
                        deferred[0][0]()
                        deferred[0] = None
                    if w in hot and w not in agg_done:
                        window_agg(w)
                        agg_done.add(w)
                    whot = w in hot
                    hq = hT[w][:]
                    h8q = h8[w][:]

                    def gate_group(m, with_q):
                        # fp8 gate block m.  cold: DoubleRow(h0,h1) + plain
                        # h2; hot: 3 DoubleRow matmuls straight through the
                        # combined [h0..2 | q0..2] rhs
                        gp = ps.tile([128, 512], f32, tag="gg",
                                     space="PSUM", name="gp", bufs=GBUFS)
                        nc.tensor.matmul(
                            gp[:, :B],
                            lhsT=wc8[:, m, 0:2, :],
                            rhs=h8q[:, 0:2, :],
                            start=True, stop=False, perf_mode=DR)
                        if with_q:
                            nc.tensor.matmul(
                                gp[:, :B],
                                lhsT=wc8[:, m, 2:4, :],
                                rhs=h8q[:, 2:4, :],
                                start=False, stop=False, perf_mode=DR)
                            nc.tensor.matmul(
                                gp[:, :B],
                                lhsT=wc8[:, m, 4:6, :],
                                rhs=h8q[:, 4:6, :],
                                start=False, stop=True, perf_mode=DR)
                        else:
                            nc.tensor.matmul(
                                gp[:, :B],
                                lhsT=wc8[:, m, 2, :],
                                rhs=h8q[:, 2, :],
                                start=False, stop=True)
                        return gp

                    def n_group_bf(j, path, rhs):
                        # bf16 n-gate block j for one path (0=ih/q, 1=hh/h)
                        gp = ps.tile([128, 512], f32, tag="gg",
                                     space="PSUM", name="gp", bufs=GBUFS)
                        for kt in range(3):
                            nc.tensor.matmul(
                                gp[:, :B],
                                lhsT=wcn[:, path, j, kt, :],
                                rhs=rhs[:, kt, :],
                                start=(kt == 0), stop=(kt == 2))
                        return gp

                    def n_group_f8(j, path, rhs):
                        # path 0 = ih (kt 3..5), 1 = hh (kt 0..2)
                        k0 = 0 if path == 1 else 3
                        gp = ps.tile([128, 512], f32, tag="gg",
                                     space="PSUM", name="gp", bufs=GBUFS)
                        nc.tensor.matmul(
                            gp[:, :B],
                            lhsT=wc8[:, 6 + j, k0:k0 + 2, :],
                            rhs=rhs[:, k0:k0 + 2, :],
                            start=True, stop=False, perf_mode=DR)
                        nc.tensor.matmul(
                            gp[:, :B],
                            lhsT=wc8[:, 6 + j, k0 + 2, :],
                            rhs=rhs[:, k0 + 2, :],
                            start=False, stop=True)
                        return gp

                    r_sb = wk.tile([128, 3, B], bf, tag="r", bufs=WB,
                                   name="r_sb")
                    z_sb = wk.tile([128, 3, B], bf, tag="z", bufs=WB,
                                   name="z_sb")
                    for j in range(3):
                        rp = gate_group(j, whot)
                        nc.scalar.activation(r_sb[:, j, :], rp[:, :B],
                                             AF.Sigmoid, bias=bcol[:, j:j + 1],
                                             scale=1.0 / W8SCALE)
                    for j in range(3):
                        zp = gate_group(3 + j, whot)
                        nc.scalar.activation(z_sb[:, j, :], zp[:, :B],
                                             AF.Sigmoid,
                                             bias=bcol[:, 3 + j:4 + j],
                                             scale=1.0 / W8SCALE)
                    t2 = wk.tile([128, 3, B], bf, tag="t2", bufs=WB,
                                 name="t2")
                    for j in range(3):
                        if N8:
                            hp = n_group_f8(j, 1, h8q)
                        else:
                            hp = n_group_bf(j, 1, hq)
                        if whot:
                            # t1 = (gh_n*16 + 16*b_hh_n) * r, then
                            # t2 = (gi_n*16 + 16*b_ih_n) + t1 (all x16; the
                            # tanh un-scales); cold: t2 = t1, bias rides tanh
                            t1 = wk.tile([128, B], bf, tag="t1", bufs=WB,
                                         name="t1")
                            nc.vector.scalar_tensor_tensor(
                                out=t1[:], in0=hp[:, :B],
                                scalar=bcol[:, 6 + j:7 + j],
                                in1=r_sb[:, j, :],
                                op0=Alu.add, op1=Alu.mult)
                            if N8:
                                ip = n_group_f8(j, 0, h8q)
                            else:
                                ip = n_group_bf(j, 0, qtiles[w])
                            nc.vector.scalar_tensor_tensor(
                                out=t2[:, j, :], in0=ip[:, :B],
                                scalar=bcol[:, 9 + j:10 + j],
                                in1=t1[:],
                                op0=Alu.add, op1=Alu.add)
                        else:
                            nc.vector.scalar_tensor_tensor(
                                out=t2[:, j, :], in0=hp[:, :B],
                                scalar=bcol[:, 6 + j:7 + j],
                                in1=r_sb[:, j, :],
                                op0=Alu.add, op1=Alu.mult)

                    def tail_math(w=w, hq=hq, h8q=h8q, t2=t2, z_sb=z_sb,
                                  k=k, whot=whot):
                        n_sb = wk.tile([128, 3, B], bf, tag="n", bufs=WB,
                                       name="n_sb")
                        d_sb = wk.tile([128, 3, B], bf, tag="d", bufs=WB,
                                       name="d_sb")
                        e_sb = wk.tile([128, 3, B], bf, tag="e", bufs=WB,
                                       name="e_sb")
                        tsc = 1.0 / W8SCALE if N8 else 1.0
                        if whot:
                            # bias already folded into t2 by the stt
                            nc.scalar.activation(n_sb[:], t2[:], AF.Tanh,
                                                 scale=tsc)
                        else:
                            for j in range(3):
                                nc.scalar.activation(
                                    n_sb[:, j, :], t2[:, j, :],
                                    AF.Tanh, bias=bcol[:, 12 + j:13 + j],
                                    scale=tsc)
                        nc.vector.tensor_sub(out=d_sb[:], in0=hq, in1=n_sb[:])
                        nc.vector.tensor_mul(out=e_sb[:], in0=z_sb[:],
                                             in1=d_sb[:])
                        nc.vector.tensor_add(out=hq, in0=n_sb[:],
                                             in1=e_sb[:])
                        if k < ITERS - 1:
                            if H8DMA:
                                # fp8 shadow refresh as a casting SBUF->SBUF
                                # DMA (on the DMA engines, not an ALU)
                                nc.gpsimd.dma_start(out=h8q[:, 0:3, :],
                                                    in_=hq)
                            else:
                                nc.vector.tensor_copy(h8q[:, 0:3, :], hq)
                        else:
                            nc.sync.dma_start(out=OUT[w], in_=hq)

                    def tail_export(w=w, hq=hq, src_cbs=src_cbs,
                                    fire=(w == last_src_w),
                                    agin=agin if k + 1 < ITERS else None,
                                    agout=agout if k + 1 < ITERS else None,
                                    P1=P1 if k + 1 < ITERS else None):
                        nonlocal spos
                        for cb in src_cbs.get(w, []):
                            tp = ps.tile([128, B], bf, tag="gg",
                                         space="PSUM", name="tp",
                                         bufs=GBUFS)
                            for kt in range(3):
                                nc.tensor.transpose(
                                    tp[:, kt * 128:(kt + 1) * 128],
                                    hq[:, kt, cb * 128:(cb + 1) * 128],
                                    idn[:])
                            rm = wk.tile([128, D], agdt, tag="rm", bufs=4)
                            nc.vector.tensor_copy(rm[:], tp[:])
                            nc.gpsimd.indirect_dma_start(
                                out=agin[:],
                                out_offset=bass.IndirectOffsetOnAxis(
                                    ap=sidx[:, spos:spos + 1], axis=0),
                                in_=rm[:], in_offset=None,
                                bounds_check=P1 - 1, oob_is_err=False)
                            spos += 1
                        if fire:
                            nc.gpsimd.collective_compute(
                                "AllGather", Alu.bypass,
                                replica_groups=[list(range(NC))],
                                ins=[agin[:].opt()],
                                outs=[agout[:].opt()])

                    if w in src_cbs:
                        # source windows: h' math is emitted eagerly so the
                        # tanh/DVE chain overlaps this window's own matmul
                        # drain; only the transpose/scatter half is deferred
                        # behind the next window's matmuls.
                        tail_math()
                        this_tail = tail_export
                    else:
                        def this_tail(tm=tail_math, te=tail_export):
                            tm()
                            te()
                    if deferred[0] is not None:
                        deferred[0][0]()
                    deferred[0] = (this_tail, w)

                # next iteration's gathers go on the gpsimd queue only after
                # every scatter/cast of this iteration, so their wait on the
                # (in-flight) AllGather never blocks this iteration's tail
                # work; the collective is long done when these execute.
                if k + 1 < ITERS and last_src_w is not None:
                    pend[k + 1] = emit_gathers(k + 1, agout)

            if deferred[0] is not None:
                deferred[0][0]()
                deferred[0] = None
    nc.compile()
    return nc


# ----------------------------------------------------------------------------
# host packing + entry point
# ----------------------------------------------------------------------------

def pack_inputs(sched, c, V, conv_weight, w_ih, w_hh, b_ih, b_hh):
    V = np.asarray(V, dtype=np.float32)
    Wcat = np.concatenate([np.asarray(conv_weight) @ np.asarray(w_ih).T,
                           np.asarray(w_hh).T], axis=0).astype(np.float32)
    b_ih = np.asarray(b_ih, dtype=np.float32)
    b_hh = np.asarray(b_hh, dtype=np.float32)

    grows = _global_rows_of(c)
    hl = np.zeros((LROWS, D), dtype=np.float32)
    valid = grows >= 0
    hl[valid] = V[grows[valid]]
    # VT0[p, w, kt, j] = h[w*B + j, kt*128 + p]  (partition-major)
    vt0f = np.ascontiguousarray(
        hl.reshape(WPC, B, 3, 128).transpose(3, 0, 2, 1))
    vt0 = vt0f.astype(BF16)
    vt08 = vt0f.astype(BF16).astype(FP8)
    # WC8[p, m, kt6, :]: kt6 0..2 -> hh weights (Wcat rows 384+), kt6 3..5
    # -> ih weights (Wcat rows 0..383); m = output block 0..8, x16 scale
    NM = 9 if N8 else 6
    wc8 = np.zeros((128, NM, 6, 128), dtype=np.float32)
    wcn = np.zeros((128, 2, 3, 3, 128), dtype=np.float32)
    for path in range(2):
        for kt in range(3):
            krow = (path * 3 + kt) * 128
            kt6 = kt if path == 1 else 3 + kt
            for m in range(NM):
                wc8[:, m, kt6, :] = (
                    Wcat[krow:krow + 128, m * 128:(m + 1) * 128] * W8SCALE)
            for j in range(3):
                mm = 6 + j
                wcn[:, path, j, kt, :] = \
                    Wcat[krow:krow + 128, mm * 128:(mm + 1) * 128]
    bsum = b_ih + b_hh
    bl = np.zeros((15, 128), dtype=np.float32)
    for m in range(6):
        bl[m] = bsum[m * 128:(m + 1) * 128]
    for j in range(3):
        bhn = b_hh[768 + j * 128: 768 + (j + 1) * 128]
        bin_ = b_ih[768 + j * 128: 768 + (j + 1) * 128]
        if N8:
            bl[6 + j] = bhn * W8SCALE
            bl[9 + j] = bin_ * W8SCALE
        else:
            bl[6 + j] = bhn
            bl[9 + j] = bin_
        bl[12 + j] = bin_
    bc = np.ascontiguousarray(bl.T)  # [128, 15]

    gidx = np.concatenate([it["gidx"][c] for it in sched], axis=0)  # [NGB,128]
    smat = np.concatenate([it["smat"][c] for it in sched], axis=0)  # [NGB,128,B]
    sc = [tab[c] for it in sched for tab in it["scat_tables"]]
    sidx = (np.stack(sc, axis=0) if sc else np.zeros((1, 128), np.int32))

    out = {
        "VT0": vt0,
        "VT08": vt08,
        "VF": V.astype(BF16),
        "WC8": wc8.astype(FP8),
        "BCOL": bc.astype(np.float32),
        "IDN": np.eye(128, dtype=np.float32).astype(BF16),
        "GIDX": np.ascontiguousarray(gidx.T).astype(np.int32),
        "SIDX": np.ascontiguousarray(sidx.T).astype(np.int32),
        # SMAT[p, blk, j] = smat[blk, p, j]  (partition-major)
        "SMAT": np.ascontiguousarray(smat.transpose(1, 0, 2)).astype(BF16),
    }
    if not N8:
        out["WCN"] = wcn.astype(BF16)
    return out


def unpack_output(results):
    out = np.zeros((N, D), dtype=np.float32)
    for c in range(NC):
        o = np.asarray(results[c]["OUT"], dtype=np.float32)  # [WPC,128,3,B]
        hl = o.transpose(0, 3, 2, 1).reshape(LROWS, D)
        grows = _global_rows_of(c)
        valid = grows >= 0
        out[grows[valid]] = hl[valid]
    return out


_CACHE = {}


def _install_profile_hook():
    """The agent image lacks ``antenv.axon_hooks``; shim it so
    run_bass_kernel_spmd(trace=True) can capture NTFF profiles."""
    import types
    try:
        from antenv.axon_hooks import get_axon_ntff_profile_hook  # noqa: F401
        return True
    except ImportError:
        pass
    try:
        import antenv
        from trn_agent_boot.trn_boot import _ntff_profile_via_ctypes
        hook = _ntff_profile_via_ctypes("/opt/axon/libaxon_pjrt.so")
        mod = types.ModuleType("antenv.axon_hooks")
        mod._hook = hook
        mod.set_axon_ntff_profile_hook = lambda h: setattr(mod, "_hook", h)
        mod.get_axon_ntff_profile_hook = lambda: mod._hook
        sys.modules["antenv.axon_hooks"] = mod
        antenv.axon_hooks = mod
        return hook is not None
    except Exception:
        return False


def kernel(V, E, conv_weight, w_ih, w_hh, b_ih, b_hh, _want_results=False):
    from concourse import bass_utils
    E_np = np.asarray(E)
    sched = build_schedule(E_np)
    key = tuple((it["nblk_total"], it["P"], tuple(it["hotwins"]),
                 tuple(it["src"]["swin_cb"]) if it["src"] else ())
                for it in sched)
    if key not in _CACHE:
        _CACHE[key] = build_bass(sched)
    nc = _CACHE[key]
    in_maps = [pack_inputs(sched, c, V, conv_weight, w_ih, w_hh, b_ih, b_hh)
               for c in range(NC)]
    trace = os.environ.get("KERNEL_TRACE", "0") == "1"
    if trace:
        trace = _install_profile_hook()
        # artifact upload to the fish bucket is unavailable here; stub it
        bass_utils.upload_artifacts = lambda tmpdir: "local://" + str(tmpdir)
    res = bass_utils.run_bass_kernel_spmd(
        nc, in_maps, core_ids=list(range(NC)), trace=trace,
        tmpdir=os.environ.get("KERNEL_TMPDIR"))
    out = unpack_output(res.results).astype(np.float32)
    if _want_results:
        return out, res
    return out
